# revision 1
# baseline (speedup 1.0000x reference)
"""Trainium2 Bass kernel for BrickVectorEdgeModel (GNN message passing).

Reference computation (per batch element b of 8):
  f  = relu(relu(x @ Wa + ba) @ Wb + bb)            # node MLP, x: [128, 256]
  e[i, j] = cat(f[j], f[i])                         # pairwise concat
  h1 = relu(e @ Wca + bca)                          # decomposed: G[j] + H[i]
  h2 = relu(h1 @ Wcb + bcb)
  h3 = relu(h2 @ Wcc + bcc)
  out[i, j] = h3 @ Wo + bo                          # [128, 128, 2]

Sharding: data-parallel over batch, one batch element per NeuronCore (8 cores).

Device kernel works in transposed activation layout [feat (partitions), cols]:
each layer is out_T[fo, col] = sum_k W[k, fo] * act_T[k, col], i.e.
matmul(psum, lhsT=W_chunk, rhs=actT_chunk), so activations never need an
on-chip transpose. The first edge layer is decomposed:
  h1_T[:, (i, j)] = relu(G_T[:, j] + (H_T[:, i] + bca))
which is a per-partition-scalar broadcast add + relu (one tensor_scalar op
per 128x128 block) instead of a [16384, 1024] x [1024, 512] matmul.

All matmuls run in bf16 with fp32 PSUM accumulation (measured end-to-end
scale-relative absmax error vs the fp32 reference: ~0.7%).
"""

import numpy as np
import ml_dtypes

import concourse.bass as bass
import concourse.mybir as mybir
import concourse.tile as tile
from concourse import bacc
from concourse.bass_utils import run_bass_kernel_spmd

BF16 = mybir.dt.bfloat16
F32 = mybir.dt.float32

B = 8          # batch == number of cores
N = 128        # bricks per model (nodes)
D_IN = 256     # input feature dim
H = 512        # hidden dim
KA = D_IN // 128   # 2 input-feature chunks
C = H // 128       # 4 hidden-feature chunks
IG = 4             # i-values per group (4 * 128 cols = 512 = one PSUM bank)
NG = N // IG       # 32 groups

# Stashed by kernel() for harnesses that want profiling info (exec_time_ns
# is populated when BASS_TRACE=1 and the NTFF hook is available).
LAST_RESULTS = None


def _build_nc() -> bass.Bass:
    # Bacc (not raw Bass): its compile pass legalizes multi-wait sync_info
    # into forms walrus codegen accepts (raw Bass + Tile hits "Too many
    # sync wait commands" in CoreV2GenImpl setupSyncWait).
    nc = bacc.Bacc("TRN2", target_bir_lowering=False)

    # Inputs (host pre-packs: weights [K, F] -> [128, K//128, F] bf16,
    # biases [F] -> [128, F//128] f32, x -> x.T packed the same way).
    xT = nc.dram_tensor("xT", [128, KA, N], BF16, kind="ExternalInput")
    Wa = nc.dram_tensor("Wa", [128, KA, H], BF16, kind="ExternalInput")
    Wb = nc.dram_tensor("Wb", [128, C, H], BF16, kind="ExternalInput")
    Wcaj = nc.dram_tensor("Wcaj", [128, C, H], BF16, kind="ExternalInput")
    Wcai = nc.dram_tensor("Wcai", [128, C, H], BF16, kind="ExternalInput")
    Wcb = nc.dram_tensor("Wcb", [128, C, H], BF16, kind="ExternalInput")
    Wcc = nc.dram_tensor("Wcc", [128, C, H], BF16, kind="ExternalInput")
    Wo = nc.dram_tensor("Wo", [128, C, 2], BF16, kind="ExternalInput")
    baT = nc.dram_tensor("baT", [1, H], BF16, kind="ExternalInput")
    bbT = nc.dram_tensor("bbT", [1, H], BF16, kind="ExternalInput")
    bcaT = nc.dram_tensor("bcaT", [1, H], BF16, kind="ExternalInput")
    bcb = nc.dram_tensor("bcb", [128, C], F32, kind="ExternalInput")
    bcc = nc.dram_tensor("bcc", [128, C], F32, kind="ExternalInput")
    bo = nc.dram_tensor("bo", [2, 1], F32, kind="ExternalInput")

    # Output in transposed layout [2, i, j]; host transposes to [i, j, 2].
    out = nc.dram_tensor("out", [2, N, N], F32, kind="ExternalOutput")

    relu = mybir.ActivationFunctionType.Relu
    ident = mybir.ActivationFunctionType.Identity
    add_op = mybir.AluOpType.add
    max_op = mybir.AluOpType.max

    with tile.TileContext(nc) as tc:
        with (
            tc.tile_pool(name="consts", bufs=1) as consts,
            tc.tile_pool(name="work", bufs=4) as work,
            tc.tile_pool(name="outp", bufs=6) as outp,
            tc.tile_pool(name="psmid", bufs=8, space="PSUM") as psmid,
        ):
            # ---- load constants -------------------------------------------------
            # All loads on the sync engine's DMA queue. Measured alternatives
            # that LOSE: round-robin across sync/scalar/gpsimd queues (+7µs),
            # and moving even just the G/H + late weights to the scalar queue
            # (+4µs — scalar-issued DMAs perturb the ACT drain schedule).
            def load(ap, shape, dt, split=False):
                t = consts.tile(shape, dt, tag=ap.name + "_sb")
                if split:
                    h = shape[1] // 2
                    nc.sync.dma_start(out=t[:, :h], in_=ap[:, :h])
                    nc.sync.dma_start(out=t[:, h:], in_=ap[:, h:])
                else:
                    nc.sync.dma_start(out=t, in_=ap[:])
                return t

            xT_sb = load(xT, [128, KA, N], BF16)
            wa_sb = load(Wa, [128, KA, H], BF16)
            baT_sb = load(baT, [1, H], BF16)
            wb_sb = load(Wb, [128, C, H], BF16, split=True)
            bbT_sb = load(bbT, [1, H], BF16)
            wcaj_sb = load(Wcaj, [128, C, H], BF16, split=True)
            wcai_sb = load(Wcai, [128, C, H], BF16, split=True)
            bcaT_sb = load(bcaT, [1, H], BF16)
            wcb_sb = load(Wcb, [128, C, H], BF16, split=True)
            bcb_sb = load(bcb, [128, C], F32)
            wcc_sb = load(Wcc, [128, C, H], BF16, split=True)
            bcc_sb = load(bcc, [128, C], F32)
            wo_sb = load(Wo, [128, C, 2], BF16)
            bo_sb = load(bo, [2, 1], F32)

            ones_sb = consts.tile([1, N], BF16, tag="ones_sb")
            nc.vector.memset(ones_sb, 1.0)

            # ---- node MLP (tiny): f2_T = relu(Wb_T @ relu(Wa_T @ x_T + ba) + bb)
            # Each layer's 4 fo-chunks go to disjoint 128-col slices of ONE
            # psum bank so a single wide op drains the whole layer. The bias is
            # folded into the accumulation as a K=1 matmul against a ones row
            # (a start=True matmul clears has_written for the whole bank but
            # not the data, so sequential per-slice groups are safe).
            def node_layer(w_sb, in_sb, kc, out_sb, biasT_sb, func):
                pst = psmid.tile([128, C, N], F32, tag="pst")
                for fo in range(C):
                    for k in range(kc):
                        nc.tensor.matmul(
                            pst[:, fo, :], w_sb[:, k, fo * 128:(fo + 1) * 128],
                            in_sb[:, k, :],
                            start=(k == 0), stop=(k == kc - 1 and biasT_sb is None),
                        )
                    if biasT_sb is not None:
                        nc.tensor.matmul(
                            pst[:, fo, :], biasT_sb[:, fo * 128:(fo + 1) * 128],
                            ones_sb, start=False, stop=True,
                        )
                if func is None:
                    nc.scalar.copy(out_sb[:, :, :], pst)
                else:
                    nc.scalar.activation(out_sb[:, :, :], pst, func)

            f1_sb = consts.tile([128, C, N], BF16, tag="f1_sb")
            node_layer(wa_sb, xT_sb, KA, f1_sb, baT_sb, relu)
            f2_sb = consts.tile([128, C, N], BF16, tag="f2_sb")
            node_layer(wb_sb, f1_sb, C, f2_sb, bbT_sb, relu)

            # ---- G_T = Wcaj_T @ f2_T ; H'_T = Wcai_T @ f2_T + bca --------------
            # Chunk-interleaved with per-chunk drains and the group-0 h1 build
            # so the first edge matmul isn't gated on the full G/H production.
            gt_sb = consts.tile([128, C, N], BF16, tag="gt_sb")
            ht_sb = consts.tile([128, C, N], F32, tag="ht_sb")
            h1_first = work.tile([128, C, IG * N], BF16, tag="h1_sb")
            for fo in range(C):
                pst = psmid.tile([128, N], F32, tag="pst")
                for k in range(C):
                    nc.tensor.matmul(
                        pst, wcaj_sb[:, k, fo * 128:(fo + 1) * 128], f2_sb[:, k, :],
                        start=(k == 0), stop=(k == C - 1),
                    )
                nc.scalar.copy(gt_sb[:, fo, :], pst)
                pst2 = psmid.tile([128, N], F32, tag="pst")
                for k in range(C):
                    nc.tensor.matmul(
                        pst2, wcai_sb[:, k, fo * 128:(fo + 1) * 128], f2_sb[:, k, :],
                        start=(k == 0), stop=False,
                    )
                nc.tensor.matmul(
                    pst2, bcaT_sb[:, fo * 128:(fo + 1) * 128], ones_sb,
                    start=False, stop=True,
                )
                nc.scalar.copy(ht_sb[:, fo, :], pst2)
                for il in range(IG):
                    nc.vector.tensor_scalar(
                        h1_first[:, fo, il * N:(il + 1) * N],
                        gt_sb[:, fo, :],
                        ht_sb[:, fo, il:il + 1],
                        0.0, add_op, max_op,
                    )

            # ---- edge MLP over 32 groups of 4 i-values (512 cols each) ---------
            # h1-build for group g+1 is emitted mid-body (before group g's h3
            # drains) so the DVE FIFO runs it while the PE works on group g —
            # the next group's cb matmuls then start without waiting on DVE.
            def build_h1(g):
                # h1[c][:, il*128 + j] = relu(G_T[c][:, j] + H'_T[c][:, g*IG+il])
                # Split across DVE and the otherwise-idle GpSimd engine.
                t = work.tile([128, C, IG * N], BF16, tag="h1_sb")
                for c in range(C):
                    eng = nc.vector
                    for il in range(IG):
                        eng.tensor_scalar(
                            t[:, c, il * N:(il + 1) * N],
                            gt_sb[:, c, :],
                            ht_sb[:, c, g * IG + il:g * IG + il + 1],
                            0.0, add_op, max_op,
                        )
                return t

            # out_T[2, cols] = Wo_T @ h3 + bo for group g — emitted AFTER the
            # next group's cb matmuls so the PE never idles at a group
            # boundary waiting for h3 drains (the out matmuls used to block
            # ready cb work in the in-order PE queue).
            def emit_out(g, h3_sb):
                pso = psmid.tile([2, IG * N], F32, tag="pst")
                for k in range(C):
                    nc.tensor.matmul(
                        pso, wo_sb[:, k, :], h3_sb[:, k, :],
                        start=(k == 0), stop=(k == C - 1),
                    )
                o_sb = outp.tile([2, IG, N], F32, tag="o_sb")
                nc.scalar.activation(o_sb, pso, ident, bias=bo_sb)
                nc.sync.dma_start(out=out[:, g * IG:(g + 1) * IG, :], in_=o_sb)

            h1_next = h1_first
            h3_prev = None
            for g in range(NG):
                h1_sb = h1_next
                # Emit the next group's h1 build first: the DVE starts it
                # immediately (it has no deps on group g), keeping its work
                # out of the contended cc-phase window.
                if g + 1 < NG:
                    h1_next = build_h1(g + 1)

                # h2 = relu(Wcb_T @ h1 + bcb)
                h2_sb = work.tile([128, C, IG * N], BF16, tag="h2_sb")
                for fo in range(C):
                    pst = psmid.tile([128, IG * N], F32, tag="pst")
                    for k in range(C):
                        nc.tensor.matmul(
                            pst, wcb_sb[:, k, fo * 128:(fo + 1) * 128], h1_sb[:, k, :],
                            start=(k == 0), stop=(k == C - 1),
                        )
                    nc.scalar.activation(h2_sb[:, fo, :], pst, relu,
                                         bias=bcb_sb[:, fo:fo + 1])

                if h3_prev is not None:
                    emit_out(g - 1, h3_prev)

                # h3 = relu(Wcc_T @ h2 + bcc); drains split DVE/ACT for balance
                h3_sb = work.tile([128, C, IG * N], BF16, tag="h3_sb")
                for fo in range(C):
                    pst = psmid.tile([128, IG * N], F32, tag="pst")
                    for k in range(C):
                        nc.tensor.matmul(
                            pst, wcc_sb[:, k, fo * 128:(fo + 1) * 128], h2_sb[:, k, :],
                            start=(k == 0), stop=(k == C - 1),
                        )
                    if fo % 2 == 0:
                        nc.vector.tensor_scalar(
                            h3_sb[:, fo, :], pst, bcc_sb[:, fo:fo + 1], 0.0,
                            add_op, max_op,
                        )
                    else:
                        nc.scalar.activation(h3_sb[:, fo, :], pst, relu,
                                             bias=bcc_sb[:, fo:fo + 1])
                h3_prev = h3_sb

            emit_out(NG - 1, h3_prev)

    nc.compile()
    return nc


def _pack_w(w: np.ndarray) -> np.ndarray:
    """[K, F] f32 -> [128, K//128, F] bf16 so W[k, f] = packed[k % 128, k // 128, f]."""
    k, f = w.shape
    return np.ascontiguousarray(
        w.reshape(k // 128, 128, f).transpose(1, 0, 2)
    ).astype(ml_dtypes.bfloat16)


def _pack_b(b: np.ndarray) -> np.ndarray:
    """[F] f32 -> [128, F//128] f32 so b[f] = packed[f % 128, f // 128]."""
    return np.ascontiguousarray(b.reshape(-1, 128).T).astype(np.float32)


def kernel(brick_vectors, Wa, ba, Wb, bb, Wca, bca, Wcb, bcb, Wcc, bcc, Wo, bo):
    global LAST_RESULTS
    brick_vectors = np.asarray(brick_vectors, dtype=np.float32)

    shared = {
        "Wa": _pack_w(np.asarray(Wa)),
        "Wb": _pack_w(np.asarray(Wb)),
        "Wcaj": _pack_w(np.asarray(Wca)[:H]),
        "Wcai": _pack_w(np.asarray(Wca)[H:]),
        "Wcb": _pack_w(np.asarray(Wcb)),
        "Wcc": _pack_w(np.asarray(Wcc)),
        "Wo": _pack_w(np.asarray(Wo)),
        "baT": np.asarray(ba).reshape(1, H).astype(ml_dtypes.bfloat16),
        "bbT": np.asarray(bb).reshape(1, H).astype(ml_dtypes.bfloat16),
        "bcaT": np.asarray(bca).reshape(1, H).astype(ml_dtypes.bfloat16),
        "bcb": _pack_b(np.asarray(bcb)),
        "bcc": _pack_b(np.asarray(bcc)),
        "bo": np.asarray(bo, dtype=np.float32).reshape(2, 1),
    }

    in_maps = []
    for b in range(B):
        xt = _pack_w(brick_vectors[b].T.astype(np.float32))  # [128, KA, N] bf16
        in_maps.append({"xT": xt, **shared})

    nc = _build_nc()
    res = run_bass_kernel_spmd(nc, in_maps, core_ids=list(range(B)))
    LAST_RESULTS = res

    out = np.empty((B, N, N, 2), dtype=np.float32)
    for b in range(B):
        out[b] = res.results[b]["out"].transpose(1, 2, 0)
    return out

